# revision 7
# baseline (speedup 1.0000x reference)
"""Causal MQA kernel for Trainium2, SPMD over 8 NeuronCores.

Sharding: tensor-parallel over query heads (16 heads / 8 cores = 2 heads per
core); the single shared KV head is replicated (classic MQA TP layout). Each
core computes the full kv projection, its 2 query heads' projection, causal
attention for those heads, and writes its [B, T, 256] slice of the output.
The host concatenates slices along the channel dim (no device collectives).

Device algorithm (per core, per batch):
  - x arrives transposed (xT[b] = x[b].T, [C, T]) so the projections emit
    kT/vT/qT in [head_dim, T] layout directly.
  - S^T[k, q] = matmul(lhsT=kT_blk, rhs=qT_chunk): keys on partitions.
  - P^T = exp(S^T / sqrt(hd)) with no max-subtraction (scores are O(1) for
    this problem's 0.02-scaled weights, exp cannot overflow); causal mask
    applied multiplicatively after exp on diagonal blocks only.
  - y^T [d, q] accumulates in PSUM via matmul(lhsT=V_blk [keys, d], rhs=P^T);
    softmax denominators accumulate via matmul(lhsT=ones [keys, 1]).
  - y = (y^T * broadcast(1/sums)).T via PE transpose, DMA'd out.
All matmuls run as float32r (full-rate fp32 mode on the PE array at N=512).
"""

import math
from contextlib import ExitStack

import numpy as np

import concourse.bass as bass
import concourse.mybir as mybir
import concourse.tile as tile
from concourse import bacc
from concourse.bass_utils import run_bass_kernel_spmd
from concourse.masks import make_identity

F32 = mybir.dt.float32
F32R = mybir.dt.float32r
P = 128  # partitions
HD = 128  # head dim
QC = 512  # query-chunk width (one fp32 PSUM bank)
KGRP = 2  # key tiles per exp group
N_CORES = 8


def r(ap):
    return ap.bitcast(F32R)


def build_nc(B, T, C, HPC, waves=2):
    """Build the per-core Bass program. HPC = query heads per core."""
    NQC = T // QC  # query chunks
    NKT = T // P  # key tiles
    NCC = C // P  # contraction chunks
    CCW = NCC // waves  # c-chunks per load wave
    KTQ = QC // P  # key tiles per query-chunk width (4)
    inv_sqrt_hd = 1.0 / math.sqrt(HD)

    nc = bacc.Bacc("TRN2", target_bir_lowering=False, debug=False,
                   num_devices=N_CORES)
    xT = nc.dram_tensor("xT", [B, C, T], F32, kind="ExternalInput").ap()
    wq_t = nc.dram_tensor("wq_t", [C, HPC * HD], F32, kind="ExternalInput").ap()
    wkv_t = nc.dram_tensor("wkv_t", [C, 2 * HD], F32, kind="ExternalInput").ap()
    y = nc.dram_tensor("y", [B, T, HPC * HD], F32, kind="ExternalOutput").ap()

    with tile.TileContext(nc) as tc, ExitStack() as ctx, \
            nc.allow_low_precision(reason="float32r tiles (~19-bit mantissa) feed the PE; accumulation stays fp32 in PSUM"):
        consts = ctx.enter_context(tc.tile_pool(name="consts", bufs=1))
        identity = consts.tile([P, P], F32)
        make_identity(nc, identity)
        ones_f32 = consts.tile([P, 1], F32)
        nc.gpsimd.memset(ones_f32, 1.0)
        ones_col = consts.tile([P, 1], F32R)
        nc.vector.tensor_copy(ones_col, ones_f32)
        ones_rf32 = consts.tile([1, P], F32)
        nc.gpsimd.memset(ones_rf32, 1.0)
        ones_row = consts.tile([1, P], F32R)
        nc.vector.tensor_copy(ones_row, ones_rf32)

        # Causal masks for the two diagonal key-tile groups of each query
        # chunk. mask[k, u, q] = 1 iff q >= k + 128*u + off  (off = 0, 256).
        masks = []
        for off in (0, KGRP * P):
            m = consts.tile([P, KGRP, QC], F32, tag=f"mask{off}")
            nc.gpsimd.memset(m, 1.0)
            nc.gpsimd.affine_select(
                out=m, in_=m,
                pattern=[[-P, KGRP], [1, QC]],
                compare_op=mybir.AluOpType.is_ge,
                fill=0.0,
                base=-off,
                channel_multiplier=-1,
            )
            masks.append(m)

        wkv_sb = consts.tile([P, NCC, 2 * HD], F32R, tag="wkv")
        nc.sync.dma_start(out=wkv_sb,
                          in_=r(wkv_t.rearrange("(cc p) d -> p cc d", p=P)))
        wq_sb = consts.tile([P, NCC, HPC * HD], F32R, tag="wq")
        nc.sync.dma_start(out=wq_sb,
                          in_=r(wq_t.rearrange("(cc p) d -> p cc d", p=P)))

        xt_pool = ctx.enter_context(tc.tile_pool(name="xt", bufs=CCW + 1))
        kT_pool = ctx.enter_context(tc.tile_pool(name="kT", bufs=1))
        vT_pool = ctx.enter_context(tc.tile_pool(name="vT", bufs=1))
        v_pool = ctx.enter_context(tc.tile_pool(name="v", bufs=1))
        qT_pool = ctx.enter_context(tc.tile_pool(name="qT", bufs=1))
        pt_pool = ctx.enter_context(tc.tile_pool(name="pt", bufs=3))
        ysum_pool = ctx.enter_context(tc.tile_pool(name="ysum", bufs=2))
        bc_pool = ctx.enter_context(tc.tile_pool(name="bc", bufs=2))
        yout_pool = ctx.enter_context(tc.tile_pool(name="yout", bufs=2))
        recip_pool = ctx.enter_context(tc.tile_pool(name="recip", bufs=2))

        # PSUM budget (8 banks): st 2x2 + y 1 + sums 1 + bc 1 + ytr 1.
        # Projection accumulators share st's slots; v-transposes share ytr's.
        st_pp = ctx.enter_context(tc.tile_pool(name="st_pp", bufs=2,
                                               space="PSUM"))
        y_pp = ctx.enter_context(tc.tile_pool(name="y_pp", bufs=1,
                                              space="PSUM"))
        sums_pp = ctx.enter_context(tc.tile_pool(name="sums_pp", bufs=1,
                                                 space="PSUM"))
        bc_pp = ctx.enter_context(tc.tile_pool(name="bc_pp", bufs=1,
                                               space="PSUM"))
        ytr_pp = ctx.enter_context(tc.tile_pool(name="ytr_pp", bufs=1,
                                                space="PSUM"))

        for b in range(B):
            # ---- projections: kT, vT, qT accumulated over c in waves ----
            kT = kT_pool.tile([P, T], F32R, tag="kT")
            vT = vT_pool.tile([P, T], F32, tag="vT")
            v_sb = v_pool.tile([P, T], F32R, tag="v")
            qT = qT_pool.tile([P, HPC, T], F32R, tag="qT")

            outs = [(kT, wkv_sb, 0), (vT, wkv_sb, 1)]
            outs += [(qT[:, h], wq_sb, h) for h in range(HPC)]
            for w in range(waves):
              with nc.named_scope(f"proj{b}w{w}"):
                ccs = list(range(w * CCW, (w + 1) * CCW))
                xts = {}
                for cc in ccs:
                    xtile = xt_pool.tile([P, T], F32R, tag="xt")
                    nc.sync.dma_start(out=xtile,
                                      in_=r(xT[b, cc * P:(cc + 1) * P, :]))
                    xts[cc] = xtile
                for dst, wsb, m in outs:
                    for n in range(T // QC):
                        ps = st_pp.tile([P, QC], F32, tag="st")
                        for i, cc in enumerate(ccs):
                            nc.tensor.matmul(
                                ps,
                                lhsT=wsb[:, cc, m * HD:(m + 1) * HD],
                                rhs=xts[cc][:, n * QC:(n + 1) * QC],
                                start=(i == 0), stop=(i == len(ccs) - 1),
                            )
                        dslc = dst[:, n * QC:(n + 1) * QC]
                        if w == 0:
                            nc.vector.tensor_copy(dslc, ps)
                        else:
                            nc.vector.tensor_add(dslc, dslc, ps)

            # ---- v into natural [t, d] layout via PE transpose ----
            with nc.named_scope(f"vtr{b}"):
              for kt in range(NKT):
                vp = ytr_pp.tile([P, HD], F32, tag="ytr")
                nc.tensor.transpose(vp, vT[:, kt * P:(kt + 1) * P], identity)
                nc.vector.tensor_copy(v_sb[:, kt * HD:(kt + 1) * HD], vp)

            # ---- attention per local head ----
            for h in range(HPC):
              with nc.named_scope(f"attn{b}h{h}"):
                for qc in range(NQC):
                    nkt = (qc + 1) * KTQ
                    ngr = nkt // KGRP
                    y_ps = y_pp.tile([P, QC], F32, tag="y")
                    s_ps = sums_pp.tile([1, QC], F32, tag="sums")
                    qrhs = qT[:, h, qc * QC:(qc + 1) * QC]
                    for g in range(ngr):
                        st = st_pp.tile([P, KGRP, QC], F32, tag="st")
                        for u in range(KGRP):
                            kt = g * KGRP + u
                            nc.tensor.matmul(
                                st[:, u], lhsT=kT[:, kt * P:(kt + 1) * P],
                                rhs=qrhs, start=True, stop=True)
                        pt = pt_pool.tile([P, KGRP, QC], F32R, tag="pt")
                        nc.scalar.activation(
                            pt, st, mybir.ActivationFunctionType.Exp,
                            scale=inv_sqrt_hd)
                        if g == 2 * qc:
                            nc.vector.tensor_mul(pt, pt, masks[0])
                        elif g == 2 * qc + 1:
                            nc.vector.tensor_mul(pt, pt, masks[1])
                        first, last = g == 0, g == ngr - 1
                        for u in range(KGRP):
                            kt = g * KGRP + u
                            prhs = pt[:, u]
                            nc.tensor.matmul(
                                y_ps, lhsT=v_sb[:, kt * HD:(kt + 1) * HD],
                                rhs=prhs,
                                start=(first and u == 0),
                                stop=(last and u == KGRP - 1))
                            nc.tensor.matmul(
                                s_ps, lhsT=ones_col, rhs=prhs,
                                start=(first and u == 0),
                                stop=(last and u == KGRP - 1))
                    # Evict y^T early (frees the accumulation bank), then
                    # normalize in SBUF once 1/sums is broadcast.
                    ysum = ysum_pool.tile([P, QC], F32, tag="ysum")
                    nc.vector.tensor_copy(ysum, y_ps)
                    rc = recip_pool.tile([1, QC], F32R, tag="recip")
                    nc.vector.reciprocal(rc, s_ps)
                    bcp = bc_pp.tile([P, QC], F32, tag="bcp")
                    nc.tensor.matmul(bcp, lhsT=ones_row, rhs=rc,
                                     start=True, stop=True)
                    bcs = bc_pool.tile([P, QC], F32, tag="bcs")
                    nc.any.tensor_copy(bcs, bcp)
                    nc.vector.tensor_mul(ysum, ysum, bcs)
                    ytr = ytr_pp.tile([P, QC], F32, tag="ytr")
                    for qt in range(KTQ):
                        nc.tensor.transpose(ytr[:, qt * P:(qt + 1) * P],
                                            ysum[:, qt * P:(qt + 1) * P],
                                            identity)
                    yo = yout_pool.tile([P, QC], F32, tag="yo")
                    nc.any.tensor_copy(yo, ytr)
                    ydst = y[b].rearrange(
                        "(nq qt p) (h d) -> nq h p qt d",
                        qt=KTQ, p=P, h=HPC)[qc, h]
                    nc.sync.dma_start(
                        out=ydst,
                        in_=yo.rearrange("p (qt d) -> p qt d", qt=KTQ))

    nc.compile()
    return nc


_cache = {}


def _get_nc(B, T, C, HPC):
    key = (B, T, C, HPC)
    if key not in _cache:
        _cache[key] = build_nc(B, T, C, HPC)
    return _cache[key]


def prepare_in_maps(x, w_kv, w_q):
    x = np.asarray(x)
    n_head = 16
    hpc = n_head // N_CORES
    xT = np.ascontiguousarray(x.transpose(0, 2, 1)).astype(np.float32)
    wkv_t = np.ascontiguousarray(np.asarray(w_kv, dtype=np.float32).T)
    in_maps = []
    for i in range(N_CORES):
        wq_sh = np.ascontiguousarray(
            np.asarray(w_q, dtype=np.float32)[i * hpc * HD:(i + 1) * hpc * HD].T)
        in_maps.append({"xT": xT, "wq_t": wq_sh, "wkv_t": wkv_t})
    return in_maps


def gather_output(results):
    return np.concatenate([results[i]["y"] for i in range(N_CORES)], axis=-1)


def kernel(x, w_kv, w_q):
    x = np.asarray(x)
    B, T, C = x.shape
    nc = _get_nc(B, T, C, 16 // N_CORES)
    in_maps = prepare_in_maps(x, w_kv, w_q)
    res = run_bass_kernel_spmd(nc, in_maps, list(range(N_CORES)))
    return gather_output(res.results)
